# revision 1
# baseline (speedup 1.0000x reference)
"""ComplexAttention Trainium2 kernel (8-core SPMD, head-parallel).

Math (per head, all on-device unless noted):
  qr,qi,kr,ki = complex projections of q,k (fp32r matmuls, T-layout [DIM,S]),
  + positional encodings (+ biases, folded host-side into pe), q-side scaled
  by DIM**-0.5 (folded host-side into qw/pe_q).
  Transposed scores: sr_T[k,q], si_T[k,q] via 4 fp32r matmuls per tile.
  m2 = sr^2 + si^2 (custom DVE square + GPSIMD add),
  scores = sqrt(m2 + eps) = exp(0.5*ln(m2+eps)) (ScalarE, one table set),
  p = exp(scores) (no max-subtraction: scores <= ~36, exp fits fp32),
  softmax denominator via ones-matmul over partitions (PE),
  attention output ar_T,ai_T = v^T @ p_T (bf16 matmuls),
  gating gar = gr*ar - gi*ai etc (GPSIMD/DVE), normalization folded to the end,
  out projection (fp32r) then * (1/denominator) (folded into PSUM evacuation).
  Output bias ob added host-side (exact; normalization commutes with W@x).

Sharding: 16 heads / 8 cores = 2 heads per core, fully communication-free.
Host transposes activations to [DIM, S] so all DMAs are contiguous.
"""

import os

os.environ.setdefault("JAX_PLATFORMS", "")

import numpy as np
import ml_dtypes

import concourse.bacc as bacc
import concourse.tile as tile
import concourse.mybir as mybir
from concourse import bass_utils
from concourse.dve_spec import Spec, Src0, Src1, sq, lower as dve_lower
from concourse.dve_uop import DveOpSpec
import concourse.dve_ops as dve_ops

F32 = mybir.dt.float32
F32R = mybir.dt.float32r
BF16 = mybir.dt.bfloat16
ALU = mybir.AluOpType
ACTF = mybir.ActivationFunctionType

B, H, S, DIM = 1, 16, 2048, 128
NCORES = 8
HPC = H // NCORES          # heads per core
SCALE = DIM ** -0.5
EPS = 1e-8 * SCALE * SCALE # eps after folding scale into q-side
NKT = S // 128             # 16 k tiles
NQC = S // 512             # 4 q chunks
GROUP = 4                  # k-tiles per ScalarE batch


def _register_dve_op(name, spec):
    for op in dve_ops.OPS:
        if op.name == name:
            return op
    opcode = dve_ops._CUSTOM_DVE_ROW_BASE + len(dve_ops.OPS)
    shas = {}
    for ver in ("v3", "v4"):
        d = DveOpSpec(name=name, opcode=opcode, uops=dve_lower(spec, ver=ver),
                      rd1_en=None)
        shas[ver] = d.sha(ver)
    op = dve_ops.DveOp(name, spec, subdim=False, uops_sha=shas)
    dve_ops.OPS.append(op)
    dve_ops._SUB_OPCODE_FOR_NAME[name] = opcode
    return op


SQ_OP = _register_dve_op("ANT_CA_SQ", Spec(
    body=sq(Src0),
    reference=lambda in0, in1, s0, s1, imm2: in0 * in0,
))


def _pin_act_table():
    """Constrain the act-table chooser so every ScalarE func we use (Ln, Exp,
    Copy) resolves to the single natural_log_exp_and_others set — otherwise
    the compiler alternates table loads (~2.7us each) between Ln and Exp."""
    from concourse.hw_specs import get_activation_tables
    tabs = get_activation_tables("gen3")
    keep = "natural_log_exp_and_others"
    assert keep in tabs
    used = {ACTF.Ln, ACTF.Exp, ACTF.Copy, ACTF.Identity}
    assert used <= tabs[keep]
    for k, v in tabs.items():
        if k != keep:
            tabs[k] = v - used


def build_nc(loop_n=None):
    _pin_act_table()
    nc = bacc.Bacc("TRN2", target_bir_lowering=False, debug=False,
                   enable_asserts=False, num_devices=NCORES)

    def din(name, shape, dtype):
        return nc.dram_tensor(name, shape, dtype, kind="ExternalInput")

    def dout(name, shape, dtype):
        return nc.dram_tensor(name, shape, dtype, kind="ExternalOutput")

    d = {}
    for n in ("q_r", "q_i", "k_r", "k_i"):
        d[n] = din(n, [HPC, DIM, S], F32R)
    for n in ("v_r", "v_i"):
        d[n] = din(n, [HPC, DIM, S], BF16)
    for n in ("peq_r", "peq_i", "pek_r", "pek_i"):
        d[n] = din(n, [HPC, DIM, S], F32)
    for n in ("wq_r", "wq_i", "wq_in", "wk_r", "wk_i", "wk_in",
              "wg_r", "wg_i", "wg_in", "wo_r", "wo_i", "wo_in"):
        d[n] = din(n, [DIM, DIM], F32R)
    for n in ("wv_r", "wv_i", "wv_in"):
        d[n] = din(n, [DIM, DIM], BF16)
    for n in ("gb_r", "gb_i"):
        d[n] = din(n, [DIM, 1], F32)
    for n in ("vb_r_w", "vb_i_w"):
        d[n] = din(n, [DIM, 128], F32)
    d["ones_col"] = din("ones_col", [DIM, 1], BF16)
    d["out_r"] = dout("out_r", [HPC, DIM, S], F32)
    d["out_i"] = dout("out_i", [HPC, DIM, S], F32)
    d["g_r"] = dout("g_r", [HPC, DIM, S], F32)
    d["g_i"] = dout("g_i", [HPC, DIM, S], F32)

    import contextlib
    with tile.TileContext(nc) as tc, contextlib.ExitStack() as ctx:
        if loop_n is not None:
            ctx.enter_context(tc.For_i(0, loop_n, 1))
        wp = ctx.enter_context(tc.tile_pool(name="wp", bufs=1))
        inp = ctx.enter_context(tc.tile_pool(name="inp", bufs=1))
        prj = ctx.enter_context(tc.tile_pool(name="prj", bufs=1))
        sbt = ctx.enter_context(tc.tile_pool(name="sbt", bufs=2))
        pp = ctx.enter_context(tc.tile_pool(name="pp", bufs=2, space="PSUM"))
        acc = ctx.enter_context(tc.tile_pool(name="acc", bufs=1, space="PSUM"))

        # ---- load weights & consts (once) ----
        w = {}
        for n in ("wq_r", "wq_i", "wq_in", "wk_r", "wk_i", "wk_in",
                  "wg_r", "wg_i", "wg_in", "wo_r", "wo_i", "wo_in"):
            w[n] = wp.tile([DIM, DIM], F32R, tag=n, name=n)
            nc.sync.dma_start(w[n][:], d[n].ap())
        for n in ("wv_r", "wv_i", "wv_in"):
            w[n] = wp.tile([DIM, DIM], BF16, tag=n, name=n)
            nc.sync.dma_start(w[n][:], d[n].ap())
        for n in ("gb_r", "gb_i"):
            w[n] = wp.tile([DIM, 1], F32, tag=n, name=n)
            nc.sync.dma_start(w[n][:], d[n].ap())
        for n in ("vb_r_w", "vb_i_w"):
            w[n] = wp.tile([DIM, 128], F32, tag=n, name=n)
            nc.sync.dma_start(w[n][:], d[n].ap())
        ones_col = wp.tile([DIM, 1], BF16, tag="ones_col")
        nc.sync.dma_start(ones_col[:], d["ones_col"].ap())
        eps_col = wp.tile([DIM, 1], F32, tag="eps_col")
        nc.vector.memset(eps_col[:], EPS)
        zero_col = wp.tile([DIM, 1], F32, tag="zero_col")
        nc.vector.memset(zero_col[:], 0.0)

        def emit_loads(h):
            raw = {}
            order = (("q_r", F32R), ("q_i", F32R), ("peq_r", F32), ("peq_i", F32),
                     ("k_r", F32R), ("k_i", F32R), ("pek_r", F32), ("pek_i", F32),
                     ("v_r", BF16), ("v_i", BF16))
            for n, dt in order:
                raw[n] = inp.tile([DIM, S], dt, tag=n, name=n)
                for half in range(2):
                    hsl = slice(half * 1024, half * 1024 + 1024)
                    nc.sync.dma_start(raw[n][:, hsl], d[n].ap()[h][:, hsl])
            return raw

        def emit_proj(h, raw):
            qrT = prj.tile([DIM, S], F32R, tag="qrT", name="qrT")
            qiT = prj.tile([DIM, S], F32R, tag="qiT", name="qiT")
            qinT = prj.tile([DIM, S], F32R, tag="qinT", name="qinT")
            krT = prj.tile([DIM, S], F32R, tag="krT", name="krT")
            kiT = prj.tile([DIM, S], F32R, tag="kiT", name="kiT")
            grT = prj.tile([DIM, S], F32, tag="grT", name="grT")
            giT = prj.tile([DIM, S], F32, tag="giT", name="giT")
            vrN = prj.tile([DIM, S], BF16, tag="vrN", name="vrN")
            viN = prj.tile([DIM, S], BF16, tag="viN", name="viN")

            def proj_T(dst, wa, xa, wb, xb, pe):
                for piece in range(2):
                    psl = slice(piece * 1024, piece * 1024 + 1024)
                    pe_t = raw[pe][:, psl]
                    ps = pp.tile([DIM, 1024], F32, tag="mm", name="ps")
                    for half in range(2):
                        sl = slice(piece * 1024 + half * 512,
                                   piece * 1024 + half * 512 + 512)
                        hs = slice(half * 512, half * 512 + 512)
                        nc.tensor.matmul(ps[:, hs], w[wa][:], raw[xa][:, sl],
                                         start=True, stop=False)
                        nc.tensor.matmul(ps[:, hs], w[wb][:], raw[xb][:, sl],
                                         start=False, stop=True)
                    nc.vector.tensor_tensor(dst[:, psl], ps[:], pe_t[:],
                                            ALU.add)

            proj_T(qrT, "wq_r", "q_r", "wq_in", "q_i", "peq_r")
            proj_T(qiT, "wq_i", "q_r", "wq_r", "q_i", "peq_i")
            for piece in range(2):
                psl = slice(piece * 1024, piece * 1024 + 1024)
                nc.vector.tensor_scalar_mul(qinT[:, psl], qiT[:, psl], -1.0)
            proj_T(krT, "wk_r", "k_r", "wk_in", "k_i", "pek_r")
            proj_T(kiT, "wk_i", "k_r", "wk_r", "k_i", "pek_i")

            for piece in range(2):
                for (dst, wa, wb, gb) in ((grT, "wg_r", "wg_in", "gb_r"),
                                          (giT, "wg_i", "wg_r", "gb_i")):
                    ps = pp.tile([DIM, 1024], F32, tag="mm", name="ps")
                    for half in range(2):
                        sl = slice(piece * 1024 + half * 512,
                                   piece * 1024 + half * 512 + 512)
                        hs = slice(half * 512, half * 512 + 512)
                        nc.tensor.matmul(ps[:, hs], w[wa][:], raw["q_r"][:, sl],
                                         start=True, stop=False)
                        nc.tensor.matmul(ps[:, hs], w[wb][:], raw["q_i"][:, sl],
                                         start=False, stop=True)
                    psl = slice(piece * 1024, piece * 1024 + 1024)
                    nc.vector.tensor_scalar_add(dst[:, psl], ps[:], w[gb][:])
            nc.sync.dma_start(d["g_r"].ap()[h], grT[:])
            nc.sync.dma_start(d["g_i"].ap()[h], giT[:])

            for piece in range(2):
                for (dst, wa, wb, vb) in ((vrN, "wv_r", "wv_in", "vb_r_w"),
                                          (viN, "wv_i", "wv_r", "vb_i_w")):
                    ps = pp.tile([DIM, 1024], F32, tag="mm", name="ps")
                    for stl in range(8):
                        st = piece * 8 + stl
                        ssl = slice(st * 128, st * 128 + 128)
                        dsl = slice(stl * 128, stl * 128 + 128)
                        nc.tensor.matmul(ps[:, dsl], raw["v_r"][:, ssl],
                                         w[wa][:], start=True, stop=False)
                        nc.tensor.matmul(ps[:, dsl], raw["v_i"][:, ssl],
                                         w[wb][:], start=False, stop=True)
                    psl = slice(piece * 1024, piece * 1024 + 1024)
                    nc.vector.tensor_tensor(
                        dst[:, psl].rearrange("p (s f) -> p s f", s=8),
                        ps[:].rearrange("p (s f) -> p s f", s=8),
                        w[vb][:].rearrange("p (o f) -> p o f", o=1).broadcast_to(
                            [DIM, 8, 128]),
                        ALU.add)
            return dict(qrT=qrT, qiT=qiT, qinT=qinT, krT=krT, kiT=kiT,
                        grT=grT, giT=giT, vrN=vrN, viN=viN)

        attnv_pending = []

        def drain_attnv(limit):
            while len(attnv_pending) > limit:
                attnv_pending.pop(0)()

        def emit_attn_qc(h, qc, p):
            qsl = slice(qc * 512, qc * 512 + 512)
            ar_ps = acc.tile([DIM, 512], F32, tag="ar", name="ar_ps")
            ai_ps = acc.tile([DIM, 512], F32, tag="ai", name="ai_ps")
            sm_ps = acc.tile([1, 512], F32, tag="sums", name="sm_ps")

            for g in range(NKT // GROUP):
                m2g = sbt.tile([DIM, GROUP * 512], F32, tag="m2g", name="m2g")
                pg = sbt.tile([DIM, GROUP * 512], BF16, tag="pg", bufs=3,
                              name="pg")
                for j in range(GROUP):
                    kt = g * GROUP + j
                    ksl = slice(kt * 128, kt * 128 + 128)
                    sc = pp.tile([DIM, 1024], F32, tag="mm", name="sc")
                    nc.tensor.matmul(sc[:, :512], p["krT"][:, ksl],
                                     p["qrT"][:, qsl], start=True, stop=False)
                    nc.tensor.matmul(sc[:, 512:], p["krT"][:, ksl],
                                     p["qinT"][:, qsl], start=True, stop=False)
                    nc.tensor.matmul(sc[:, :512], p["kiT"][:, ksl],
                                     p["qiT"][:, qsl], start=False, stop=True)
                    nc.tensor.matmul(sc[:, 512:], p["kiT"][:, ksl],
                                     p["qrT"][:, qsl], start=False, stop=True)
                    tt = sbt.tile([DIM, 1024], F32, tag="tt", name="tt", bufs=3)
                    nc.vector._custom_dve(SQ_OP, out=tt[:], in0=sc[:])
                    jsl = slice(j * 512, j * 512 + 512)
                    nc.gpsimd.tensor_tensor(m2g[:, jsl], tt[:, :512],
                                            tt[:, 512:], ALU.add)
                nc.scalar.activation(m2g[:], m2g[:], ACTF.Ln, bias=eps_col[:])
                nc.scalar.activation(m2g[:], m2g[:], ACTF.Exp,
                                     bias=zero_col[:], scale=0.5)
                nc.scalar.activation(pg[:], m2g[:], ACTF.Exp, bias=zero_col[:])

                def attnv(g=g, pg=pg, ar_ps=ar_ps, ai_ps=ai_ps, sm_ps=sm_ps):
                    for j in range(GROUP):
                        kt = g * GROUP + j
                        ktile = slice(kt * 128, kt * 128 + 128)
                        psl = pg[:, j * 512:j * 512 + 512]
                        st, sp = (kt == 0), (kt == NKT - 1)
                        nc.tensor.matmul(ar_ps[:], p["vrN"][:, ktile], psl,
                                         start=st, stop=sp)
                        nc.tensor.matmul(ai_ps[:], p["viN"][:, ktile], psl,
                                         start=st, stop=sp)
                        nc.tensor.matmul(sm_ps[:], ones_col[:], psl,
                                         start=st, stop=sp)
                attnv_pending.append(attnv)
                drain_attnv(2)
            drain_attnv(0)

            # evacuate PSUM promptly so the next q-chunk's accumulators are free
            car = sbt.tile([DIM, 512], F32, tag="car", bufs=2, name="car")
            nc.scalar.copy(car[:], ar_ps[:])
            cai = sbt.tile([DIM, 512], F32, tag="cai", bufs=2, name="cai")
            nc.scalar.copy(cai[:], ai_ps[:])
            rrow = sbt.tile([1, 512], F32, tag="rrow", bufs=2, name="rrow")
            nc.vector.reciprocal_approx_fast(rrow[:], sm_ps[:])
            return car, cai, rrow

        def emit_gate_out(h, qc, p, car, cai, rrow):
            qsl = slice(qc * 512, qc * 512 + 512)
            bc = sbt.tile([DIM, 512], F32, tag="bc", bufs=1, name="bc")
            nc.gpsimd.partition_broadcast(bc[:], rrow[:])
            gm1 = sbt.tile([DIM, 512], F32, tag="gm1", bufs=1, name="gm1")
            gm2 = sbt.tile([DIM, 512], F32, tag="gm2", bufs=1, name="gm2")
            gm3 = sbt.tile([DIM, 512], F32, tag="gm1", bufs=1, name="gm3")
            gm4 = sbt.tile([DIM, 512], F32, tag="gm2", bufs=1, name="gm4")
            nc.gpsimd.tensor_tensor(gm1[:], p["grT"][:, qsl], car[:], ALU.mult)
            nc.gpsimd.tensor_tensor(gm2[:], p["giT"][:, qsl], cai[:], ALU.mult)
            nc.gpsimd.tensor_tensor(gm3[:], p["grT"][:, qsl], cai[:], ALU.mult)
            nc.gpsimd.tensor_tensor(gm4[:], p["giT"][:, qsl], car[:], ALU.mult)
            gar = sbt.tile([DIM, 512], F32R, tag="gar", bufs=1, name="gar")
            nc.vector.tensor_tensor(gar[:], gm1[:], gm2[:], ALU.subtract)
            gai = sbt.tile([DIM, 512], F32R, tag="gai", bufs=1, name="gai")
            nc.vector.tensor_tensor(gai[:], gm3[:], gm4[:], ALU.add)
            for (wa, wb, dname) in (("wo_r", "wo_in", "out_r"),
                                    ("wo_i", "wo_r", "out_i")):
                op_ps = acc.tile([DIM, 512], F32, tag="op", name="op_ps")
                nc.tensor.matmul(op_ps[:], w[wa][:], gar[:],
                                 start=True, stop=False)
                nc.tensor.matmul(op_ps[:], w[wb][:], gai[:],
                                 start=False, stop=True)
                oc = sbt.tile([DIM, 512], F32, tag="car", name="oc", bufs=2)
                nc.vector.scalar_tensor_tensor(oc[:], op_ps[:], 1.0, bc[:],
                                               ALU.mult, ALU.mult)
                nc.sync.dma_start(d[dname].ap()[h][:, qsl], oc[:])

        # --- emission schedule: pipeline heads and defer gating one chunk ---
        raw0 = emit_loads(0)
        p0 = emit_proj(0, raw0)
        projs = {0: p0}
        pending = None
        for h in range(HPC):
            for qc in range(NQC):
                if h + 1 < HPC and qc == 2:
                    raw1 = emit_loads(h + 1)
                gate_args = emit_attn_qc(h, qc, projs[h])
                if pending is not None:
                    emit_gate_out(*pending)
                pending = (h, qc, projs[h]) + gate_args
                if h + 1 < HPC and qc == NQC - 1:
                    projs[h + 1] = emit_proj(h + 1, raw1)
        if pending is not None:
            emit_gate_out(*pending)

    nc.compile()
    return nc


_NC = None


def _get_nc():
    global _NC
    if _NC is None:
        _NC = build_nc()
    return _NC


def make_in_maps(inputs):
    f = {k: np.asarray(v, dtype=np.float32) for k, v in inputs.items()}
    bf16 = ml_dtypes.bfloat16

    def T(x):  # [h, S, DIM] -> [h, DIM, S] contiguous
        return np.ascontiguousarray(np.transpose(x, (0, 2, 1)))

    shared = {
        "wq_r": np.ascontiguousarray(f["qw_r"].T) * SCALE,
        "wq_i": np.ascontiguousarray(f["qw_i"].T) * SCALE,
        "wq_in": np.ascontiguousarray(-f["qw_i"].T) * SCALE,
        "wk_r": np.ascontiguousarray(f["kw_r"].T),
        "wk_i": np.ascontiguousarray(f["kw_i"].T),
        "wk_in": np.ascontiguousarray(-f["kw_i"].T),
        "wg_r": np.ascontiguousarray(f["gw_r"].T),
        "wg_i": np.ascontiguousarray(f["gw_i"].T),
        "wg_in": np.ascontiguousarray(-f["gw_i"].T),
        "wo_r": np.ascontiguousarray(f["ow_r"].T),
        "wo_i": np.ascontiguousarray(f["ow_i"].T),
        "wo_in": np.ascontiguousarray(-f["ow_i"].T),
        "wv_r": np.ascontiguousarray(f["vw_r"].T).astype(bf16),
        "wv_i": np.ascontiguousarray(f["vw_i"].T).astype(bf16),
        "wv_in": np.ascontiguousarray(-f["vw_i"].T).astype(bf16),
        "gb_r": np.ascontiguousarray(f["gb_r"][:, None]),
        "gb_i": np.ascontiguousarray(f["gb_i"][:, None]),
        "vb_r_w": np.ascontiguousarray(np.tile(f["vb_r"][None, :], (DIM, 1))),
        "vb_i_w": np.ascontiguousarray(np.tile(f["vb_i"][None, :], (DIM, 1))),
        "ones_col": np.ones((DIM, 1), dtype=bf16),
    }
    in_maps = []
    for c in range(NCORES):
        hs = slice(c * HPC, (c + 1) * HPC)
        m = dict(shared)
        m["q_r"] = T(f["q_r"][0, hs])
        m["q_i"] = T(f["q_i"][0, hs])
        m["k_r"] = T(f["k_r"][0, hs])
        m["k_i"] = T(f["k_i"][0, hs])
        m["v_r"] = T(f["v_r"][0, hs]).astype(bf16)
        m["v_i"] = T(f["v_i"][0, hs]).astype(bf16)
        m["peq_r"] = T(f["pe_q_r"][0, hs] + f["qb_r"][None, None, :]) * SCALE
        m["peq_i"] = T(f["pe_q_i"][0, hs] + f["qb_i"][None, None, :]) * SCALE
        m["pek_r"] = T(f["pe_k_r"][0, hs] + f["kb_r"][None, None, :])
        m["pek_i"] = T(f["pe_k_i"][0, hs] + f["kb_i"][None, None, :])
        in_maps.append(m)
    return in_maps


def assemble(results, inputs):
    f32 = np.float32
    ob_r = np.asarray(inputs["ob_r"], f32)
    ob_i = np.asarray(inputs["ob_i"], f32)
    outs = {}
    for name in ("out_r", "out_i", "g_r", "g_i"):
        full = np.empty((B, H, S, DIM), f32)
        for c in range(NCORES):
            r = np.asarray(results[c][name], f32)  # [HPC, DIM, S]
            full[0, c * HPC:(c + 1) * HPC] = np.transpose(r, (0, 2, 1))
        outs[name] = full
    outs["out_r"] = outs["out_r"] + ob_r[None, None, None, :]
    outs["out_i"] = outs["out_i"] + ob_i[None, None, None, :]
    return outs["out_r"], outs["out_i"], outs["g_r"], outs["g_i"]


def kernel(**inputs):
    nc = _get_nc()
    in_maps = make_in_maps(inputs)
    res = bass_utils.run_bass_kernel_spmd(nc, in_maps,
                                          core_ids=list(range(NCORES)))
    return assemble(res.results, inputs)

